# revision 32
# baseline (speedup 1.0000x reference)
"""Single-head causal attention on 8 TRN2 NeuronCores.

Problem: x[8,2048,1024] @ Wq/Wk/Wv[1024,64] -> causal softmax attention -> out[8,2048,64].
Sharding: data-parallel over batch B=8, one batch element per core; weights replicated.

Per-core design v2 (T=2048, C=1024, H=64), evolution of the 66us PE-transpose
baseline:
 - xT is produced by the DMA XBAR transpose engine (dma_start_transpose,
   16x128 tiles, ~14ns/tile) directly from HBM into per-chunk SBUF tiles
   [128, 8, 512]: this deletes the 136 PE transposes + their LDWEIGHTS
   (~28% of PE cycles) and all DVE staging copies of the old design.
   8 transpose DMAs (per t-chunk x c-half) split across the two HWDGE
   queues (sync + scalar) so delivery is ~2x faster and chunk 0 lands
   first.
 - weights pre-cast/packed to bf16 on host, loaded on the gpsimd SWDGE
   queue (keeps both HWDGE rings free for x).
 - per-chunk pipeline otherwise like v1: q|k projected together
   (stationary [Wq|Wk]) into qT/kT; v projected and PE-transposed to
   natural [s, 64+1] with a ones column for the softmax denominator.
 - scores TRANSPOSED: weiT[s,t] = kT.T@qT per (s-block, t-chunk); two
   s-blocks per [128,1024] f32 PSUM tile; exp folds the C**-0.5 scale;
   no max subtraction (scores O(1), softmax shift-invariant).
 - scores phase software-pipelined one pair ahead: PE order is
   sc(g0) sc(g1) pv(g0) sc(g2) pv(g1) ... so each PV's exp wait is
   covered by the next pair's score matmuls; the next chunk's q|k
   projection matmuls are injected before the chunk's last PV pair so
   the chunk-closing exp latency is covered too.
 - causal mask: fully-masked blocks skipped, score matmuls and PV stream
   only [lo:] of diagonal tiles, below-diagonal of the 128x128 diagonal
   zeroed by GpSimd affine_select after the exp.
 - copies kept off the Scalar engine (it is ~23us of exp, the #2
   engine): qT/kT/vts/out copies on DVE, v_c copies on GpSimd.
 - final normalization (divide by sums + transpose [65,512]) on host.
"""

import numpy as np

import concourse.bass as bass
import concourse.mybir as mybir
import concourse.tile as tile
from concourse import bacc
from concourse.masks import make_identity
from contextlib import ExitStack

P = 128
T = 2048
C = 1024
H = 64
B = 8
NC = C // P          # 8 c-tiles
NT = T // P          # 16 s/t 128-blocks
CH = 512             # t-chunk width
NCH = T // CH        # 4 chunks
BPC = CH // P        # 4 blocks per chunk
SCALE = float(C) ** -0.5
F32 = mybir.dt.float32
BF16 = mybir.dt.bfloat16
EXP = mybir.ActivationFunctionType.Exp
N_WARM = 96          # PE warmup transposes (flip HAM clock gate to 2.4GHz)
DEBUG_DUMP = False   # add DRAM dumps of chunk-0 intermediates (debug only)
NW_DBG = 2
dbg_w_tiles = []


def build_nc():
    nc = bacc.Bacc(None, target_bir_lowering=False)
    # x pre-cast to bf16 on host: halves the HBM read (the kernel would cast
    # to bf16 on-chip anyway, so numerics are identical)
    x = nc.dram_tensor("xb16", [T, C], BF16, kind="ExternalInput")
    # weights pre-cast to bf16 and TRANSPOSED on host (W^T [3H, C]); the
    # xbar transpose DMA lands them directly in the stationary layout
    wall_d = nc.dram_tensor("WallT", [3 * H, C], BF16, kind="ExternalInput")
    out_d = nc.dram_tensor("outT", [H + 1, T], F32, kind="ExternalOutput")

    with tile.TileContext(nc) as tc, ExitStack() as ctx:
        consts = ctx.enter_context(tc.tile_pool(name="consts", bufs=1))
        xtp = ctx.enter_context(tc.tile_pool(name="xtp", bufs=1))
        persist = ctx.enter_context(tc.tile_pool(name="persist", bufs=1))
        wei = ctx.enter_context(tc.tile_pool(name="wei", bufs=8))
        vtsp = ctx.enter_context(tc.tile_pool(name="vtsp", bufs=2))
        fin = ctx.enter_context(tc.tile_pool(name="fin", bufs=2))
        # PSUM: 8 banks total; ppj 2 + psc 2x2 + pout 1 + pvn 1 = 8.
        ppj = ctx.enter_context(tc.tile_pool(name="ppj", bufs=2, space="PSUM"))
        psc = ctx.enter_context(tc.tile_pool(name="psc", bufs=2, space="PSUM"))
        pout = ctx.enter_context(tc.tile_pool(name="pout", bufs=1, space="PSUM"))
        pvn = ctx.enter_context(tc.tile_pool(name="pvn", bufs=1, space="PSUM"))

        # NOTE: every persistent tile gets a DISTINCT tag. Untagged tiles in a
        # pool share ONE ring of `bufs` buffers — with bufs=1 they all alias
        # the same address and the tile framework chains WAR semaphores
        # through them, serializing the whole kernel (found the hard way).

        # ---- ALL of xT is produced by DMA XBAR transposes
        # (xt_c[tb][p, jc, t] = x[tb*CH + t, jc*128 + p]), as half-chunk
        # slabs on the sync queue, EMITTED FIRST, before ANY SBUF compute.
        # Scheduling facts learned on HW:
        #  * the XBAR is a single shared unit — two concurrent transpose DMA
        #    streams interleave inside it and corrupt each other's 16x128
        #    tiles, so ALL transpose DMAs go on ONE queue;
        #  * the tile framework cannot range-analyze a transpose DMA's
        #    magic-encoded write AP, so each one conservatively WAITS for all
        #    previously emitted SBUF work (later ops don't wait on it though)
        #    — hence: emit them before everything else;
        #  * the scheduler serializes all DMA transfers on one modeled
        #    DMA_ENGINES resource with a ~2us hop per queue switch — one
        #    queue, consumption order;
        #  * the first DMA_TRANSPOSE pays a ~1.3-4us cold cost (xbar table
        #    setup) — absorb it on a 32-byte dummy.
        xt_c = [xtp.tile([P, NC, CH], BF16, tag=f"xt{tb}", name=f"xt{tb}")
                for tb in range(NCH)]
        # dummy transpose first (absorbs the xbar cold cost); the sync queue
        # carries ONLY transposes — a DIRECT2D->TRANSPOSE mode switch on the
        # queue costs ~2-3us and ANY extra DMA costs its slot in the global
        # serial chain, so the weights ride the SAME xbar stream: the host
        # stores W^T [3H, C] and the xbar transpose lands it directly in the
        # stationary layout wall_sb[p, jc, h] = W[jc*128+p, h].
        xbarwarm = consts.tile([P, 16], BF16, tag="xbarwarm", name="xbarwarm")
        nc.sync.dma_start_transpose(out=xbarwarm, in_=x[0:16, 0:P])
        wall_sb = consts.tile([P, NC, 3 * H], BF16, tag="wall_sb")

        def xslab(tb, h):
            nc.sync.dma_start_transpose(
                out=xt_c[tb][:, h * NC // 2 : (h + 1) * NC // 2, :],
                in_=x[tb * CH : (tb + 1) * CH, h * C // 2 : (h + 1) * C // 2],
            )

        # weights between the two chunk-0 halves: B0's jc 0-3 matmuls can
        # then start right after [s0a, wallT] instead of after full chunk 0
        xslab(0, 0)
        nc.sync.dma_start_transpose(out=wall_sb[:, :, :], in_=wall_d[:, :])
        xslab(0, 1)
        for tb in range(1, NCH):
            for h in range(2):
                xslab(tb, h)

        # ---- Pool queue: memsets + identity (no DMAs here — an early SWDGE
        # on this queue was observed to stall it for ~12us)
        warm_b = consts.tile([P, P], BF16, tag="warm_b")
        nc.gpsimd.memset(warm_b, 1.0)
        ident_f = consts.tile([P, P], F32, tag="ident_f")
        make_identity(nc, ident_f)
        ident_b = consts.tile([P, P], BF16, tag="ident_b")
        nc.vector.tensor_copy(out=ident_b, in_=ident_f)

        # ---- PE warmup: dummy transposes on the memset tile keep the PE busy
        # from ~6.5us (engine preamble) until chunk 0 lands; the HAM clock
        # gate needs ~3us of sustained PE activity to reach 2.4GHz and drops
        # back whenever the PE idles. Distinct column slots of one tile -> no
        # WAW semaphores between the first 8, so they run back to back.
        wt = pvn.tile([P, 2 * CH], BF16, tag="vt")
        for i in range(N_WARM):
            nc.tensor.transpose(wt[:, (i % NC) * P : (i % NC + 1) * P], warm_b, warm_b)

        # per-chunk persistent projections (separate tiles -> no cross-chunk WAR)
        qT_c = [persist.tile([H, CH], BF16, tag=f"qT{tb}", name=f"qT{tb}") for tb in range(NCH)]
        kT_c = [persist.tile([H, CH], BF16, tag=f"kT{tb}", name=f"kT{tb}") for tb in range(NCH)]
        v_c = [persist.tile([P, BPC, H + 1], BF16, tag=f"v{tb}", name=f"v{tb}") for tb in range(NCH)]
        for tb in range(NCH):
            nc.gpsimd.memset(v_c[tb][:, :, H : H + 1], 1.0)  # denominator column

        vts_all = [None] * NCH

        def stage_B_qk(tb):
            """q|k projection matmuls only (stationary [Wq|Wk])"""
            pqk = ppj.tile([P, CH], F32, tag="pj", name=f"pqk{tb}")
            for jc in range(NC):
                nc.tensor.matmul(pqk, lhsT=wall_sb[:, jc, 0 : 2 * H],
                                 rhs=xt_c[tb][:, jc, :],
                                 start=(jc == 0), stop=(jc == NC - 1))
            return pqk

        def stage_B_rest(tb, pqk):
            """projection copies + v matmuls + vts copy"""
            nc.vector.tensor_copy(out=qT_c[tb], in_=pqk[0:H, :])
            nc.vector.tensor_copy(out=kT_c[tb], in_=pqk[H : 2 * H, :])
            pv = ppj.tile([P, CH], F32, tag="pj", name=f"pv{tb}")
            for jc in range(NC):
                nc.tensor.matmul(pv[0:H, :], lhsT=wall_sb[:, jc, 2 * H : 3 * H],
                                 rhs=xt_c[tb][:, jc, :],
                                 start=(jc == 0), stop=(jc == NC - 1))
            vts = vtsp.tile([H, CH], BF16, tag="vt")
            nc.vector.tensor_copy(out=vts, in_=pv[0:H, :])
            vts_all[tb] = vts

        def stage_vt(tb):
            """small transposes to v natural [s, 64] + Pool copy to v_c"""
            vts = vts_all[tb]
            pvn_t = pvn.tile([P, 2 * CH], BF16, tag="vt")
            for tt in range(BPC):
                nc.tensor.transpose(pvn_t[:, tt * H : (tt + 1) * H],
                                    vts[:, tt * P : (tt + 1) * P],
                                    ident_b[0:H, 0:H])
            # (gpsimd cannot read PSUM — this copy must stay on DVE)
            nc.vector.tensor_copy(out=v_c[tb][:, :, 0:H], in_=pvn_t[:, 0 : BPC * H])

        def stage_C(tb, inject=None):
            """scores (2 s-blocks per [128,1024] f32 tile) + exp + mask + PV,
            software-pipelined one pair ahead so PV's exp wait is covered by
            the next pair's score matmuls; `inject` (next chunk's q|k
            matmuls) is emitted before the last PV pair."""
            po = pout.tile([H + 1, CH], F32, tag="po")
            nsb = (tb + 1) * BPC
            npairs = nsb // 2

            def emit_sc(g):
                pair = (2 * g, 2 * g + 1)
                ps = psc.tile([P, 2 * CH], F32, tag="sc")
                los = []
                for m, si in enumerate(pair):
                    lo = max(0, (si - tb * BPC) * P)
                    los.append(lo)
                    nc.tensor.matmul(
                        ps[:, m * CH + lo : (m + 1) * CH],
                        lhsT=kT_c[si // BPC][:, (si % BPC) * P : (si % BPC + 1) * P],
                        rhs=qT_c[tb][:, lo:CH],
                        start=True, stop=True,
                    )
                return ps, pair, los

            def emit_exp_pv(state, g):
                ps, pair, los = state
                w = wei.tile([P, 2 * CH], BF16, tag="w")
                if DEBUG_DUMP and tb == 0:
                    dbg_w_tiles.append(w)
                # diagonal pairs: split the exp per block — the region between
                # the two blocks' [lo:] slices is unwritten PSUM garbage, and
                # the split also shortens each PV's exp wait
                diag = pair[0] >= tb * BPC
                if not diag:
                    nc.scalar.activation(out=w[:, 0 : 2 * CH],
                                         in_=ps[:, 0 : 2 * CH],
                                         func=EXP, scale=SCALE)
                if g == 2 * tb:
                    # first diagonal pair: emit the v transposes here so they
                    # run in the PV-waits-exp slot of the PE FIFO
                    stage_vt(tb)
                for m, si in enumerate(pair):
                    lo = los[m]
                    if diag:
                        nc.scalar.activation(out=w[:, m * CH + lo : (m + 1) * CH],
                                             in_=ps[:, m * CH + lo : (m + 1) * CH],
                                             func=EXP, scale=SCALE)
                    if si >= tb * BPC:  # diagonal block: zero below-diagonal (t < s)
                        nc.gpsimd.affine_select(
                            out=w[:, m * CH + lo : m * CH + lo + P],
                            in_=w[:, m * CH + lo : m * CH + lo + P],
                            compare_op=mybir.AluOpType.is_ge,
                            fill=0.0,
                            base=0,
                            # keep where (col - row) >= 0
                            pattern=[[1, P]],
                            channel_multiplier=-1,
                        )
                    # close the accumulation group one pair early on the
                    # last chunk (stop is sim bookkeeping, free on HW) so the
                    # finished [0:256] columns may be read out early
                    last_ch = tb == NCH - 1
                    nc.tensor.matmul(po[:, lo:CH], lhsT=v_c[si // BPC][:, si % BPC, :],
                                     rhs=w[:, m * CH + lo : (m + 1) * CH],
                                     start=(g == 0 and m == 0),
                                     stop=(g == npairs - (2 if last_ch else 1)
                                           and m == 1) or
                                          (g == npairs - 1 and m == 1),
                                     skip_group_check=(last_ch and
                                                       g == npairs - 1))

            state = emit_sc(0)
            os_early = [None]
            for g in range(npairs):
                nxt = emit_sc(g + 1) if g + 1 < npairs else None
                if g == npairs - 1 and inject is not None:
                    inject()
                if tb == NCH - 1 and g == npairs - 1:
                    # cols [0:256] of po are final (the last pair only writes
                    # [256:]); copy on DVE + store them while the closing
                    # exp->PV chain runs
                    os_early[0] = fin.tile([H + 1, CH], F32, tag="ot", name="os_last")
                    nc.vector.tensor_copy(out=os_early[0][:, 0:256],
                                          in_=po[:, 0:256])
                    nc.sync.dma_start(out=out_d[:, tb * CH : tb * CH + 256],
                                      in_=os_early[0][:, 0:256])
                emit_exp_pv(state, g)
                state = nxt

            if tb == NCH - 1:
                os_ = os_early[0]
                # second half of the split output (first half was emitted
                # before the last pair; cols [0:256] were final by then)
                nc.scalar.copy(out=os_[:, 256:CH], in_=po[:, 256:CH])
                nc.sync.dma_start(out=out_d[:, tb * CH + 256 : (tb + 1) * CH],
                                  in_=os_[:, 256:CH])
            else:
                os_ = fin.tile([H + 1, CH], F32, tag="ot")
                nc.vector.tensor_copy(out=os_, in_=po)
                # out stores on the sync queue: on the scalar queue they
                # head-of-line-block the exp stream behind their (serially
                # modeled) DMA transfer slot
                nc.sync.dma_start(out=out_d[:, tb * CH : (tb + 1) * CH], in_=os_)

        # ---- main pipeline
        pqk = stage_B_qk(0)
        stage_B_rest(0, pqk)
        hold = [None]

        def make_inject(tb_next):
            def inject():
                hold[0] = stage_B_qk(tb_next)
            return inject

        for tb in range(NCH):
            stage_C(tb, inject=make_inject(tb + 1) if tb + 1 < NCH else None)
            if tb + 1 < NCH:
                stage_B_rest(tb + 1, hold[0])

        if DEBUG_DUMP:
            dq = nc.dram_tensor("dbg_q", [H, CH], BF16, kind="ExternalOutput")
            dk = nc.dram_tensor("dbg_k", [H, CH], BF16, kind="ExternalOutput")
            dv = nc.dram_tensor("dbg_v", [P, BPC, H + 1], BF16, kind="ExternalOutput")
            dw = nc.dram_tensor("dbg_w", [P, NW_DBG, 2 * CH], BF16, kind="ExternalOutput")
            nc.sync.dma_start(out=dq[:, :], in_=qT_c[0])
            nc.sync.dma_start(out=dk[:, :], in_=kT_c[0])
            nc.sync.dma_start(out=dv[:, :, :], in_=v_c[0])
            for i, wt_ in enumerate(dbg_w_tiles[:NW_DBG]):
                nc.sync.dma_start(out=dw[:, i, :], in_=wt_)
    return nc


_NC_CACHE = []


def _get_nc():
    if not _NC_CACHE:
        nc = build_nc()
        nc.finalize()  # bacc compile: register allocation, DCE
        _NC_CACHE.append(nc)
    return _NC_CACHE[0]


def make_in_maps(inputs):
    import ml_dtypes
    x = np.ascontiguousarray(
        np.asarray(inputs["x"], dtype=np.float32).astype(ml_dtypes.bfloat16))
    wq = np.asarray(inputs["Wq"], dtype=np.float32)
    wk = np.asarray(inputs["Wk"], dtype=np.float32)
    wv = np.asarray(inputs["Wv"], dtype=np.float32)
    # host-side: W^T [3H, C] bf16; the on-chip xbar transpose delivers the
    # stationary layout [p, jc, 3H]
    wallT = np.ascontiguousarray(
        np.concatenate([wq, wk, wv], axis=1).T.astype(ml_dtypes.bfloat16))
    return [{"xb16": np.ascontiguousarray(x[b]), "WallT": wallT} for b in range(B)]


def kernel(**inputs):
    from concourse.bass_utils import run_bass_kernel_spmd

    nc = _get_nc()
    res = run_bass_kernel_spmd(nc, make_in_maps(inputs), core_ids=list(range(B)))
    return postprocess([res.results[b]["outT"] for b in range(B)])


def postprocess(outTs):
    outs = []
    for oT in outTs:
        outs.append((oT[0:H, :] / oT[H : H + 1, :]).T.astype(np.float32))
    return np.stack(outs, axis=0)


if __name__ == "__main__":
    import os
    os.makedirs("/tmp/neffdir3", exist_ok=True)
    from concourse.bass_utils import compile_bass_kernel

    nc = _get_nc()
    print("build OK, instructions:",
          sum(len(bb.instructions) for bb in nc.m.functions[0].blocks))
    print("COMPILED:", compile_bass_kernel(nc, "/tmp/neffdir3"))


# revision 33
# speedup vs baseline: 1.0217x; 1.0217x over previous
"""Single-head causal attention on 8 TRN2 NeuronCores.

Problem: x[8,2048,1024] @ Wq/Wk/Wv[1024,64] -> causal softmax attention -> out[8,2048,64].
Sharding: data-parallel over batch B=8, one batch element per core; weights replicated.

Per-core design v2 (T=2048, C=1024, H=64), evolution of the 66us PE-transpose
baseline:
 - xT is produced by the DMA XBAR transpose engine (dma_start_transpose,
   16x128 tiles, ~14ns/tile) directly from HBM into per-chunk SBUF tiles
   [128, 8, 512]: this deletes the 136 PE transposes + their LDWEIGHTS
   (~28% of PE cycles) and all DVE staging copies of the old design.
   8 transpose DMAs (per t-chunk x c-half) split across the two HWDGE
   queues (sync + scalar) so delivery is ~2x faster and chunk 0 lands
   first.
 - weights pre-cast/packed to bf16 on host, loaded on the gpsimd SWDGE
   queue (keeps both HWDGE rings free for x).
 - per-chunk pipeline otherwise like v1: q|k projected together
   (stationary [Wq|Wk]) into qT/kT; v projected and PE-transposed to
   natural [s, 64+1] with a ones column for the softmax denominator.
 - scores TRANSPOSED: weiT[s,t] = kT.T@qT per (s-block, t-chunk); two
   s-blocks per [128,1024] f32 PSUM tile; exp folds the C**-0.5 scale;
   no max subtraction (scores O(1), softmax shift-invariant).
 - scores phase software-pipelined one pair ahead: PE order is
   sc(g0) sc(g1) pv(g0) sc(g2) pv(g1) ... so each PV's exp wait is
   covered by the next pair's score matmuls; the next chunk's q|k
   projection matmuls are injected before the chunk's last PV pair so
   the chunk-closing exp latency is covered too.
 - causal mask: fully-masked blocks skipped, score matmuls and PV stream
   only [lo:] of diagonal tiles, below-diagonal of the 128x128 diagonal
   zeroed by GpSimd affine_select after the exp.
 - copies kept off the Scalar engine (it is ~23us of exp, the #2
   engine): qT/kT/vts/out copies on DVE, v_c copies on GpSimd.
 - final normalization (divide by sums + transpose [65,512]) on host.
"""

import numpy as np

import concourse.bass as bass
import concourse.mybir as mybir
import concourse.tile as tile
from concourse import bacc
from concourse.masks import make_identity
from contextlib import ExitStack

P = 128
T = 2048
C = 1024
H = 64
B = 8
NC = C // P          # 8 c-tiles
NT = T // P          # 16 s/t 128-blocks
CH = 512             # t-chunk width
NCH = T // CH        # 4 chunks
BPC = CH // P        # 4 blocks per chunk
SCALE = float(C) ** -0.5
F32 = mybir.dt.float32
BF16 = mybir.dt.bfloat16
EXP = mybir.ActivationFunctionType.Exp
N_WARM = 72          # PE warmup transposes (flip HAM clock gate to 2.4GHz)
DEBUG_DUMP = False   # add DRAM dumps of chunk-0 intermediates (debug only)
NW_DBG = 2
dbg_w_tiles = []


def build_nc():
    nc = bacc.Bacc(None, target_bir_lowering=False)
    # x pre-cast to bf16 on host: halves the HBM read (the kernel would cast
    # to bf16 on-chip anyway, so numerics are identical)
    x = nc.dram_tensor("xb16", [T, C], BF16, kind="ExternalInput")
    # weights pre-cast to bf16 and TRANSPOSED on host (W^T [3H, C]); the
    # xbar transpose DMA lands them directly in the stationary layout
    wall_d = nc.dram_tensor("WallT", [3 * H, C], BF16, kind="ExternalInput")
    out_d = nc.dram_tensor("outT", [H + 1, T], F32, kind="ExternalOutput")

    with tile.TileContext(nc) as tc, ExitStack() as ctx:
        consts = ctx.enter_context(tc.tile_pool(name="consts", bufs=1))
        xtp = ctx.enter_context(tc.tile_pool(name="xtp", bufs=1))
        persist = ctx.enter_context(tc.tile_pool(name="persist", bufs=1))
        wei = ctx.enter_context(tc.tile_pool(name="wei", bufs=8))
        vtsp = ctx.enter_context(tc.tile_pool(name="vtsp", bufs=2))
        fin = ctx.enter_context(tc.tile_pool(name="fin", bufs=2))
        # PSUM: 8 banks total; ppj 2 + psc 2x2 + pout 1 + pvn 1 = 8.
        ppj = ctx.enter_context(tc.tile_pool(name="ppj", bufs=2, space="PSUM"))
        psc = ctx.enter_context(tc.tile_pool(name="psc", bufs=2, space="PSUM"))
        pout = ctx.enter_context(tc.tile_pool(name="pout", bufs=1, space="PSUM"))
        pvn = ctx.enter_context(tc.tile_pool(name="pvn", bufs=1, space="PSUM"))

        # NOTE: every persistent tile gets a DISTINCT tag. Untagged tiles in a
        # pool share ONE ring of `bufs` buffers — with bufs=1 they all alias
        # the same address and the tile framework chains WAR semaphores
        # through them, serializing the whole kernel (found the hard way).

        # ---- ALL of xT is produced by DMA XBAR transposes
        # (xt_c[tb][p, jc, t] = x[tb*CH + t, jc*128 + p]), as half-chunk
        # slabs on the sync queue, EMITTED FIRST, before ANY SBUF compute.
        # Scheduling facts learned on HW:
        #  * the XBAR is a single shared unit — two concurrent transpose DMA
        #    streams interleave inside it and corrupt each other's 16x128
        #    tiles, so ALL transpose DMAs go on ONE queue;
        #  * the tile framework cannot range-analyze a transpose DMA's
        #    magic-encoded write AP, so each one conservatively WAITS for all
        #    previously emitted SBUF work (later ops don't wait on it though)
        #    — hence: emit them before everything else;
        #  * the scheduler serializes all DMA transfers on one modeled
        #    DMA_ENGINES resource with a ~2us hop per queue switch — one
        #    queue, consumption order;
        #  * the first DMA_TRANSPOSE pays a ~1.3-4us cold cost (xbar table
        #    setup) — absorb it on a 32-byte dummy.
        xt_c = [xtp.tile([P, NC, CH], BF16, tag=f"xt{tb}", name=f"xt{tb}")
                for tb in range(NCH)]
        # dummy transpose first (absorbs the xbar cold cost); the sync queue
        # carries ONLY transposes — a DIRECT2D->TRANSPOSE mode switch on the
        # queue costs ~2-3us and ANY extra DMA costs its slot in the global
        # serial chain, so the weights ride the SAME xbar stream: the host
        # stores W^T [3H, C] and the xbar transpose lands it directly in the
        # stationary layout wall_sb[p, jc, h] = W[jc*128+p, h].
        xbarwarm = consts.tile([P, 16], BF16, tag="xbarwarm", name="xbarwarm")
        nc.sync.dma_start_transpose(out=xbarwarm, in_=x[0:16, 0:P])
        wall_sb = consts.tile([P, NC, 3 * H], BF16, tag="wall_sb")

        def xslab(tb, h):
            nc.sync.dma_start_transpose(
                out=xt_c[tb][:, h * NC // 2 : (h + 1) * NC // 2, :],
                in_=x[tb * CH : (tb + 1) * CH, h * C // 2 : (h + 1) * C // 2],
            )

        # weights between the two chunk-0 halves: B0's jc 0-3 matmuls can
        # then start right after [s0a, wallT] instead of after full chunk 0
        xslab(0, 0)
        nc.sync.dma_start_transpose(out=wall_sb[:, :, :], in_=wall_d[:, :])
        xslab(0, 1)
        for tb in range(1, NCH):
            for h in range(2):
                xslab(tb, h)

        # ---- Pool queue: memsets + identity (no DMAs here — an early SWDGE
        # on this queue was observed to stall it for ~12us)
        warm_b = consts.tile([P, P], BF16, tag="warm_b")
        nc.gpsimd.memset(warm_b, 1.0)
        ident_f = consts.tile([P, P], F32, tag="ident_f")
        make_identity(nc, ident_f)
        ident_b = consts.tile([P, P], BF16, tag="ident_b")
        nc.vector.tensor_copy(out=ident_b, in_=ident_f)

        # ---- PE warmup: dummy transposes on the memset tile keep the PE busy
        # from ~6.5us (engine preamble) until chunk 0 lands; the HAM clock
        # gate needs ~3us of sustained PE activity to reach 2.4GHz and drops
        # back whenever the PE idles. Distinct column slots of one tile -> no
        # WAW semaphores between the first 8, so they run back to back.
        wt = pvn.tile([P, 2 * CH], BF16, tag="vt")
        for i in range(N_WARM):
            nc.tensor.transpose(wt[:, (i % NC) * P : (i % NC + 1) * P], warm_b, warm_b)

        # per-chunk persistent projections (separate tiles -> no cross-chunk WAR)
        qT_c = [persist.tile([H, CH], BF16, tag=f"qT{tb}", name=f"qT{tb}") for tb in range(NCH)]
        kT_c = [persist.tile([H, CH], BF16, tag=f"kT{tb}", name=f"kT{tb}") for tb in range(NCH)]
        v_c = [persist.tile([P, BPC, H + 1], BF16, tag=f"v{tb}", name=f"v{tb}") for tb in range(NCH)]
        for tb in range(NCH):
            nc.gpsimd.memset(v_c[tb][:, :, H : H + 1], 1.0)  # denominator column

        vts_all = [None] * NCH

        def stage_B_qk(tb):
            """q|k projection matmuls only (stationary [Wq|Wk])"""
            pqk = ppj.tile([P, CH], F32, tag="pj", name=f"pqk{tb}")
            for jc in range(NC):
                nc.tensor.matmul(pqk, lhsT=wall_sb[:, jc, 0 : 2 * H],
                                 rhs=xt_c[tb][:, jc, :],
                                 start=(jc == 0), stop=(jc == NC - 1))
            return pqk

        def stage_B_rest(tb, pqk):
            """projection copies + v matmuls + vts copy"""
            nc.vector.tensor_copy(out=qT_c[tb], in_=pqk[0:H, :])
            nc.vector.tensor_copy(out=kT_c[tb], in_=pqk[H : 2 * H, :])
            pv = ppj.tile([P, CH], F32, tag="pj", name=f"pv{tb}")
            for jc in range(NC):
                nc.tensor.matmul(pv[0:H, :], lhsT=wall_sb[:, jc, 2 * H : 3 * H],
                                 rhs=xt_c[tb][:, jc, :],
                                 start=(jc == 0), stop=(jc == NC - 1))
            vts = vtsp.tile([H, CH], BF16, tag="vt")
            nc.vector.tensor_copy(out=vts, in_=pv[0:H, :])
            vts_all[tb] = vts

        def stage_vt(tb):
            """small transposes to v natural [s, 64] + Pool copy to v_c"""
            vts = vts_all[tb]
            pvn_t = pvn.tile([P, 2 * CH], BF16, tag="vt")
            for tt in range(BPC):
                nc.tensor.transpose(pvn_t[:, tt * H : (tt + 1) * H],
                                    vts[:, tt * P : (tt + 1) * P],
                                    ident_b[0:H, 0:H])
            # (gpsimd cannot read PSUM — this copy must stay on DVE)
            nc.vector.tensor_copy(out=v_c[tb][:, :, 0:H], in_=pvn_t[:, 0 : BPC * H])

        def stage_C(tb, inject=None):
            """scores (2 s-blocks per [128,1024] f32 tile) + exp + mask + PV,
            software-pipelined one pair ahead so PV's exp wait is covered by
            the next pair's score matmuls; `inject` (next chunk's q|k
            matmuls) is emitted before the last PV pair."""
            po = pout.tile([H + 1, CH], F32, tag="po")
            nsb = (tb + 1) * BPC
            npairs = nsb // 2

            def emit_sc(g):
                pair = (2 * g, 2 * g + 1)
                ps = psc.tile([P, 2 * CH], F32, tag="sc")
                los = []
                for m, si in enumerate(pair):
                    lo = max(0, (si - tb * BPC) * P)
                    los.append(lo)
                    nc.tensor.matmul(
                        ps[:, m * CH + lo : (m + 1) * CH],
                        lhsT=kT_c[si // BPC][:, (si % BPC) * P : (si % BPC + 1) * P],
                        rhs=qT_c[tb][:, lo:CH],
                        start=True, stop=True,
                    )
                return ps, pair, los

            def emit_exp_pv(state, g):
                ps, pair, los = state
                w = wei.tile([P, 2 * CH], BF16, tag="w")
                if DEBUG_DUMP and tb == 0:
                    dbg_w_tiles.append(w)
                # diagonal pairs: split the exp per block — the region between
                # the two blocks' [lo:] slices is unwritten PSUM garbage, and
                # the split also shortens each PV's exp wait
                diag = pair[0] >= tb * BPC
                if not diag:
                    nc.scalar.activation(out=w[:, 0 : 2 * CH],
                                         in_=ps[:, 0 : 2 * CH],
                                         func=EXP, scale=SCALE)
                if g == 2 * tb:
                    # first diagonal pair: emit the v transposes here so they
                    # run in the PV-waits-exp slot of the PE FIFO
                    stage_vt(tb)
                for m, si in enumerate(pair):
                    lo = los[m]
                    if diag:
                        nc.scalar.activation(out=w[:, m * CH + lo : (m + 1) * CH],
                                             in_=ps[:, m * CH + lo : (m + 1) * CH],
                                             func=EXP, scale=SCALE)
                    if si >= tb * BPC:  # diagonal block: zero below-diagonal (t < s)
                        nc.gpsimd.affine_select(
                            out=w[:, m * CH + lo : m * CH + lo + P],
                            in_=w[:, m * CH + lo : m * CH + lo + P],
                            compare_op=mybir.AluOpType.is_ge,
                            fill=0.0,
                            base=0,
                            # keep where (col - row) >= 0
                            pattern=[[1, P]],
                            channel_multiplier=-1,
                        )
                    # close the accumulation group one pair early on the
                    # last chunk (stop is sim bookkeeping, free on HW) so the
                    # finished [0:256] columns may be read out early
                    last_ch = tb == NCH - 1
                    nc.tensor.matmul(po[:, lo:CH], lhsT=v_c[si // BPC][:, si % BPC, :],
                                     rhs=w[:, m * CH + lo : (m + 1) * CH],
                                     start=(g == 0 and m == 0),
                                     stop=(g == npairs - (2 if last_ch else 1)
                                           and m == 1) or
                                          (g == npairs - 1 and m == 1),
                                     skip_group_check=(last_ch and
                                                       g == npairs - 1))

            state = emit_sc(0)
            os_early = [None]
            for g in range(npairs):
                nxt = emit_sc(g + 1) if g + 1 < npairs else None
                if g == npairs - 1 and inject is not None:
                    inject()
                if tb == NCH - 1 and g == npairs - 1:
                    # cols [0:256] of po are final (the last pair only writes
                    # [256:]); copy on DVE + store them while the closing
                    # exp->PV chain runs
                    os_early[0] = fin.tile([H + 1, CH], F32, tag="ot", name="os_last")
                    nc.vector.tensor_copy(out=os_early[0][:, 0:256],
                                          in_=po[:, 0:256])
                    nc.sync.dma_start(out=out_d[:, tb * CH : tb * CH + 256],
                                      in_=os_early[0][:, 0:256])
                emit_exp_pv(state, g)
                state = nxt

            if tb == NCH - 1:
                os_ = os_early[0]
                # second half of the split output (first half was emitted
                # before the last pair; cols [0:256] were final by then)
                nc.scalar.copy(out=os_[:, 256:CH], in_=po[:, 256:CH])
                nc.sync.dma_start(out=out_d[:, tb * CH + 256 : (tb + 1) * CH],
                                  in_=os_[:, 256:CH])
            else:
                os_ = fin.tile([H + 1, CH], F32, tag="ot")
                nc.vector.tensor_copy(out=os_, in_=po)
                # out stores on the sync queue: on the scalar queue they
                # head-of-line-block the exp stream behind their (serially
                # modeled) DMA transfer slot
                nc.sync.dma_start(out=out_d[:, tb * CH : (tb + 1) * CH], in_=os_)

        # ---- main pipeline
        pqk = stage_B_qk(0)
        stage_B_rest(0, pqk)
        hold = [None]

        def make_inject(tb_next):
            def inject():
                hold[0] = stage_B_qk(tb_next)
            return inject

        for tb in range(NCH):
            stage_C(tb, inject=make_inject(tb + 1) if tb + 1 < NCH else None)
            if tb + 1 < NCH:
                stage_B_rest(tb + 1, hold[0])

        if DEBUG_DUMP:
            dq = nc.dram_tensor("dbg_q", [H, CH], BF16, kind="ExternalOutput")
            dk = nc.dram_tensor("dbg_k", [H, CH], BF16, kind="ExternalOutput")
            dv = nc.dram_tensor("dbg_v", [P, BPC, H + 1], BF16, kind="ExternalOutput")
            dw = nc.dram_tensor("dbg_w", [P, NW_DBG, 2 * CH], BF16, kind="ExternalOutput")
            nc.sync.dma_start(out=dq[:, :], in_=qT_c[0])
            nc.sync.dma_start(out=dk[:, :], in_=kT_c[0])
            nc.sync.dma_start(out=dv[:, :, :], in_=v_c[0])
            for i, wt_ in enumerate(dbg_w_tiles[:NW_DBG]):
                nc.sync.dma_start(out=dw[:, i, :], in_=wt_)
    return nc


_NC_CACHE = []


def _get_nc():
    if not _NC_CACHE:
        nc = build_nc()
        nc.finalize()  # bacc compile: register allocation, DCE
        _NC_CACHE.append(nc)
    return _NC_CACHE[0]


def make_in_maps(inputs):
    import ml_dtypes
    x = np.ascontiguousarray(
        np.asarray(inputs["x"], dtype=np.float32).astype(ml_dtypes.bfloat16))
    wq = np.asarray(inputs["Wq"], dtype=np.float32)
    wk = np.asarray(inputs["Wk"], dtype=np.float32)
    wv = np.asarray(inputs["Wv"], dtype=np.float32)
    # host-side: W^T [3H, C] bf16; the on-chip xbar transpose delivers the
    # stationary layout [p, jc, 3H]
    wallT = np.ascontiguousarray(
        np.concatenate([wq, wk, wv], axis=1).T.astype(ml_dtypes.bfloat16))
    return [{"xb16": np.ascontiguousarray(x[b]), "WallT": wallT} for b in range(B)]


def kernel(**inputs):
    from concourse.bass_utils import run_bass_kernel_spmd

    nc = _get_nc()
    res = run_bass_kernel_spmd(nc, make_in_maps(inputs), core_ids=list(range(B)))
    return postprocess([res.results[b]["outT"] for b in range(B)])


def postprocess(outTs):
    outs = []
    for oT in outTs:
        outs.append((oT[0:H, :] / oT[H : H + 1, :]).T.astype(np.float32))
    return np.stack(outs, axis=0)


if __name__ == "__main__":
    import os
    os.makedirs("/tmp/neffdir3", exist_ok=True)
    from concourse.bass_utils import compile_bass_kernel

    nc = _get_nc()
    print("build OK, instructions:",
          sum(len(bb.instructions) for bb in nc.m.functions[0].blocks))
    print("COMPILED:", compile_bass_kernel(nc, "/tmp/neffdir3"))
